# revision 4
# baseline (speedup 1.0000x reference)
"""Trainium2 Bass kernel for nn_FAttention1d (attention with softmax over the
QUERY axis).

Reference computation (B=2, H=16, S=2048, D=64, fp32):
    att[b,h,q,k] = sum_d qry[b,h,q,d] * key[b,h,k,d]
    att += reg * I_S                      (diagonal in (q,k))
    att = softmax(att, axis=q)            (normalize over the QUERY axis)
    out[b,h,q,v] = sum_k att[b,h,q,k] * val[b,h,k,v]

Sharding: the 32 (b,h) pairs are split 4-per-core across 8 NeuronCores; the
whole S=2048 attention chain is local to a core.

Device-side layout: compute S^T = K @ Q^T with k on the partition axis, so the
q-axis softmax denominator is a free-axis reduction, and exp(S^T) tiles feed
the A^T V matmul directly as the moving operand:
    out^T[v,q] = sum_k (val[k,v] / r[k])_stationary @ exp(S^T)[k,q]
with r[k] = sum_q exp(S^T[k,q]).

Engine split (per core, ACT is the roofline at ~133us):
  PE : QK^T 512-col f16 matmuls into [128,1024] PSUM tiles; AV bf16 matmuls
       for k-tile n-2 interleaved after QK of k-tile n.
  ACT: exp straight from PSUM -> SBUF bf16 (no staging, no accum_out).
  DVE: reg*I diagonal adds into PSUM pre-exp; one tensor_scalar copy pass per
       [128,2048] exp tile in 4x_2p mode (all-SBUF 2-byte) whose accum_out
       yields the softmax denominator r; reciprocal; bf16 val-row scaling;
       PSUM->SBUF output copy.
"""

import numpy as np
import ml_dtypes
from contextlib import ExitStack

import concourse.bass as bass
import concourse.mybir as mybir
import concourse.tile as tile
from concourse import bacc
from concourse.bass_utils import run_bass_kernel_spmd

B, H, S, D = 2, 16, 2048, 64
N_CORES = 8
BH = B * H                     # 32
BH_PER_CORE = BH // N_CORES    # 4
NT = S // 128                  # 16 k-tiles of 128
F32 = mybir.dt.float32
F16 = mybir.dt.float16
BF16 = mybir.dt.bfloat16
AB = (0, 1)
AV_LAG = 2                     # AV for k-tile n-AV_LAG is emitted after QK n


def _build_kernel(nc, tc, ctx, qt, kt, vs, rg, ot):
    const_pool = ctx.enter_context(tc.tile_pool(name="const", bufs=1))
    q_pool = ctx.enter_context(tc.tile_pool(name="q", bufs=2))
    k_pool = ctx.enter_context(tc.tile_pool(name="k", bufs=2))
    v_pool = ctx.enter_context(tc.tile_pool(name="v", bufs=2))
    er_pool = ctx.enter_context(tc.tile_pool(name="er", bufs=3))
    ef_pool = ctx.enter_context(tc.tile_pool(name="ef", bufs=4))
    r_pool = ctx.enter_context(tc.tile_pool(name="r", bufs=2))
    vsc_pool = ctx.enter_context(tc.tile_pool(name="vsc", bufs=4))
    osb_pool = ctx.enter_context(tc.tile_pool(name="osb", bufs=2))
    st_pool = ctx.enter_context(tc.tile_pool(name="st", bufs=2, space="PSUM"))
    o_pool = ctx.enter_context(tc.tile_pool(name="o", bufs=1, space="PSUM"))

    rg_eye = const_pool.tile([128, 128], F32)
    nc.gpsimd.dma_start(rg_eye[:], rg[:])

    for p in range(BH_PER_CORE // 2):
        bh = (2 * p, 2 * p + 1)
        q2 = q_pool.tile([128, S], F16, tag="q2", name="q2")
        k2 = k_pool.tile([128, S], F16, tag="k2", name="k2")
        nc.sync.dma_start(q2[:, 0:512], qt[p][:, 0:512])
        nc.gpsimd.dma_start(k2[:, 0:128], kt[p][:, 0:128])
        nc.sync.dma_start(q2[:, 512:], qt[p][:, 512:])
        nc.gpsimd.dma_start(k2[:, 128:], kt[p][:, 128:])
        v_sb = [None, None]
        for s in AB:
            v_sb[s] = v_pool.tile([128, NT * 64], BF16, tag=f"v{s}", name=f"v_sb{s}")
            nc.gpsimd.dma_start(v_sb[s][:], vs[bh[s]])

        # out^T for the pair: partitions 0-63 = bh A, 64-127 = bh B
        o_ps = o_pool.tile([128, S], F32)
        r_t = [r_pool.tile([128, NT], F32, tag=f"r{s}", name=f"r{s}") for s in AB]
        r_inv = [r_pool.tile([128, NT], F32, tag=f"ri{s}", name=f"r_inv{s}")
                 for s in AB]
        ef_tiles = [[None] * NT, [None] * NT]

        def av_unit(s, m):
            # val rows scaled by 1/r, then the col-packed AV matmuls for
            # k-tile m (bh s -> out partitions 64s..64s+63)
            vsc = vsc_pool.tile([128, 64], BF16, tag=f"vsc{s}", name=f"vsc{s}")
            nc.vector.tensor_scalar_mul(
                vsc[:], v_sb[s][:, m * 64:(m + 1) * 64], r_inv[s][:, m:m + 1])
            for ch in range(4):
                cs = slice(ch * 512, (ch + 1) * 512)
                nc.tensor.matmul(
                    o_ps[64 * s:64 * s + 64, cs],
                    lhsT=vsc[:],
                    rhs=ef_tiles[s][m][:, cs],
                    start=(m == 0),
                    stop=(m == NT - 1),
                    skip_group_check=True,
                )

        for n in range(NT):
            for s in AB:
                er = er_pool.tile([128, S], BF16, tag=f"er{s}", name=f"er{s}_{n}")
                for h in range(2):
                    st = st_pool.tile([128, 1024], F32)
                    for j in range(2):
                        q0 = h * 1024 + j * 512
                        nc.tensor.matmul(
                            st[:, j * 512:(j + 1) * 512],
                            lhsT=k2[64 * s:64 * s + 64, n * 128:(n + 1) * 128],
                            rhs=q2[64 * s:64 * s + 64, q0:q0 + 512],
                            start=True,
                            stop=True,
                        )
                    # diag of S^T for k-tile n sits at q-cols 128n..128n+127;
                    # add reg*I if that block is in this half
                    if n // 8 == h:
                        c = (n % 8) * 128
                        nc.vector.tensor_add(
                            st[:, c:c + 128], st[:, c:c + 128], rg_eye[:])
                    nc.scalar.activation(
                        er[:, h * 1024:(h + 1) * 1024],
                        st[:],
                        mybir.ActivationFunctionType.Exp,
                    )
                # 4x-mode copy whose accumulator computes r = sum_q exp
                ef = ef_pool.tile([128, S], BF16, tag=f"ef{s}", name=f"ef{s}_{n}")
                ef_tiles[s][n] = ef
                nc.vector.tensor_scalar(
                    ef[:], er[:], 1.0, 0.0,
                    op0=mybir.AluOpType.mult,
                    op1=mybir.AluOpType.add,
                    accum_out=r_t[s][:, n:n + 1],
                )
                nc.vector.reciprocal_approx_fast(
                    r_inv[s][:, n:n + 1], r_t[s][:, n:n + 1])
            if n >= AV_LAG:
                for s in AB:
                    av_unit(s, n - AV_LAG)
        for m in range(NT - AV_LAG, NT):
            for s in AB:
                av_unit(s, m)

        out_sb = osb_pool.tile([128, S], F32)
        for h in range(4):
            cs = slice(h * 512, (h + 1) * 512)
            nc.vector.tensor_copy(out_sb[:, cs], o_ps[:, cs])
        for s in AB:
            nc.gpsimd.dma_start(ot[bh[s]], out_sb[64 * s:64 * s + 64, :])


_NC_CACHE = {}


def build_nc(repeats=1):
    key = repeats
    if key in _NC_CACHE:
        return _NC_CACHE[key]
    nc = bacc.Bacc("TRN2", target_bir_lowering=False, debug=False)
    qt = nc.dram_tensor("qt", [BH_PER_CORE // 2, 2 * D, S], F16, kind="ExternalInput").ap()
    kt = nc.dram_tensor("kt", [BH_PER_CORE // 2, 2 * D, S], F16, kind="ExternalInput").ap()
    vs = nc.dram_tensor("vs", [BH_PER_CORE, 128, NT * 64], BF16, kind="ExternalInput").ap()
    rg = nc.dram_tensor("rg", [128, 128], F32, kind="ExternalInput").ap()
    ot = nc.dram_tensor("ot", [BH_PER_CORE, D, S], F32, kind="ExternalOutput").ap()
    with tile.TileContext(nc) as tc, ExitStack() as ctx:
        if repeats == 1:
            _build_kernel(nc, tc, ctx, qt, kt, vs, rg, ot)
        else:
            # benchmarking mode: repeat the whole kernel body in an on-device
            # loop so per-iteration time can be extracted from wall clock
            with tc.For_i(0, repeats, 1,
                          hint_engines=(mybir.EngineType.PE,
                                        mybir.EngineType.Activation,
                                        mybir.EngineType.DVE)):
                _build_kernel(nc, tc, ctx, qt, kt, vs, rg, ot)
    nc.compile()
    _NC_CACHE[key] = nc
    return nc


def _prep_inputs(qry, key, val, reg):
    """Host-side shard + layout prep. Returns per-core input maps."""
    q = np.ascontiguousarray(np.asarray(qry, dtype=np.float32)).reshape(BH, S, D)
    k = np.ascontiguousarray(np.asarray(key, dtype=np.float32)).reshape(BH, S, D)
    v = np.ascontiguousarray(np.asarray(val, dtype=np.float32)).reshape(BH, S, D)
    rg = (np.eye(128, dtype=np.float32) * np.float32(np.asarray(reg)))

    in_maps = []
    for c in range(N_CORES):
        sl = slice(c * BH_PER_CORE, (c + 1) * BH_PER_CORE)
        qt = np.ascontiguousarray(
            q[sl].transpose(0, 2, 1).reshape(BH_PER_CORE // 2, 2 * D, S)
        ).astype(np.float16)                                          # [2, 128, S]
        kt = np.ascontiguousarray(
            k[sl].transpose(0, 2, 1).reshape(BH_PER_CORE // 2, 2 * D, S)
        ).astype(np.float16)                                          # [2, 128, S]
        vv = v[sl].reshape(BH_PER_CORE, NT, 128, D)
        vs = np.ascontiguousarray(vv.transpose(0, 2, 1, 3)).reshape(
            BH_PER_CORE, 128, NT * D).astype(ml_dtypes.bfloat16)      # [4, 128, 1024]
        in_maps.append({"qt": qt, "kt": kt, "vs": vs, "rg": rg})
    return in_maps


def kernel(qry, key, val, reg):
    nc = build_nc()
    in_maps = _prep_inputs(qry, key, val, reg)
    res = run_bass_kernel_spmd(nc, in_maps, list(range(N_CORES)))
    out = np.empty((BH, S, D), dtype=np.float32)
    for c in range(N_CORES):
        ot = res.results[c]["ot"]                                    # [4, 64, S]
        for i in range(BH_PER_CORE):
            out[c * BH_PER_CORE + i] = ot[i].T
    return out.reshape(B, H, S, D)


# revision 15
# speedup vs baseline: 1.2797x; 1.2797x over previous
"""Trainium2 Bass kernel for nn_FAttention1d (attention with softmax over the
QUERY axis).

Reference computation (B=2, H=16, S=2048, D=64, fp32):
    att[b,h,q,k] = sum_d qry[b,h,q,d] * key[b,h,k,d]
    att += reg * I_S                      (diagonal in (q,k))
    att = softmax(att, axis=q)            (normalize over the QUERY axis)
    out[b,h,q,v] = sum_k att[b,h,q,k] * val[b,h,k,v]

Sharding: the 32 (b,h) pairs are split 4-per-core across 8 NeuronCores; the
whole S=2048 attention chain is local to a core.

Device-side layout: compute S^T = K @ Q^T with k on the partition axis, so the
q-axis softmax denominator is a free-axis reduction, and exp(S^T) tiles feed
the A^T V matmul directly as the moving operand:
    out^T[v,q] = sum_k (val[k,v] / r[k])_stationary @ exp(S^T)[k,q]
with r[k] = sum_q exp(S^T[k,q]) taken from the exp pass's ACT accumulator.

Engine split (per core, ACT is the roofline at ~183us busy):
  PE : QK^T 512-col f16 matmuls into [128,1024] PSUM tiles; reg*I diagonal
       injected as an extra 128-col matmul accumulation (identity stationary,
       rg_eye moving); AV bf16 matmuls interleaved at a lag, flowing across
       pair boundaries (flat software pipeline, no per-pair barrier).
  ACT: exp straight from PSUM -> SBUF bf16 with accum_out (the only r source
       that is cheap on HW; DVE accumulator reads measured ~1us each).
  DVE: r-half adds, reciprocal, bf16 val-row scaling, PSUM->SBUF out copy.
"""

import numpy as np
import ml_dtypes
from contextlib import ExitStack

import concourse.bass as bass
import concourse.mybir as mybir
import concourse.tile as tile
from concourse import bacc
from concourse.bass_utils import run_bass_kernel_spmd

B, H, S, D = 2, 16, 2048, 64
N_CORES = 8
BH = B * H                     # 32
BH_PER_CORE = BH // N_CORES    # 4
NP = BH_PER_CORE // 2          # 2 pairs per core
NT = S // 128                  # 16 k-tiles of 128
F32 = mybir.dt.float32
F16 = mybir.dt.float16
BF16 = mybir.dt.bfloat16
AB = (0, 1)


def _build_kernel(nc, tc, ctx, qt, kt, vs, rg, rgb, ot, cfg=None):
    cfg = cfg or {}
    av_lag = cfg.get("av_lag", 2)
    diag_mode = cfg.get("diag_mode", "pe")   # pe | dve
    const_pool = ctx.enter_context(tc.tile_pool(name="const", bufs=1))
    q_pool = ctx.enter_context(tc.tile_pool(name="q", bufs=2))
    k_pool = ctx.enter_context(tc.tile_pool(name="k", bufs=2))
    v_pool = ctx.enter_context(tc.tile_pool(name="v", bufs=2))
    er_pool = ctx.enter_context(tc.tile_pool(name="er", bufs=4))
    r_pool = ctx.enter_context(tc.tile_pool(name="r", bufs=2))
    vsc_pool = ctx.enter_context(tc.tile_pool(name="vsc", bufs=4))
    osb_pool = ctx.enter_context(tc.tile_pool(name="osb", bufs=2))
    st_pool = ctx.enter_context(tc.tile_pool(name="st", bufs=2, space="PSUM"))
    o_pool = ctx.enter_context(tc.tile_pool(name="o", bufs=1, space="PSUM"))

    rg_eye = const_pool.tile([128, 128], F32)
    nc.scalar.dma_start(rg_eye[:], rg[:])
    # bf16 [identity | reg*identity] for the PE diagonal injection:
    # st += id_bf^T @ rg_bf
    eyes = const_pool.tile([128, 256], BF16)
    nc.scalar.dma_start(eyes[:], rgb[:])
    id_bf = eyes[:, 0:128]
    rg_bf = eyes[:, 128:256]

    # per-pair state
    st8 = [dict() for _ in range(NP)]

    def pair_start(p):
        d = st8[p]
        bh = (2 * p, 2 * p + 1)
        d["bh"] = bh
        q2 = q_pool.tile([128, S], F16, tag="q2", name=f"q2_{p}")
        k2 = k_pool.tile([128, S], F16, tag="k2", name=f"k2_{p}")
        nc.sync.dma_start(q2[:, 0:512], qt[p][:, 0:512])
        nc.gpsimd.dma_start(k2[:, 0:128], kt[p][:, 0:128])
        nc.sync.dma_start(q2[:, 512:], qt[p][:, 512:])
        nc.gpsimd.dma_start(k2[:, 128:], kt[p][:, 128:])
        d["q2"], d["k2"] = q2, k2
        d["v"] = []
        for s in AB:
            v = v_pool.tile([128, NT * 64], BF16, tag=f"v{s}", name=f"v{s}_{p}")
            nc.gpsimd.dma_start(v[:], vs[bh[s]])
            d["v"].append(v)
        d["r2"] = [r_pool.tile([128, 2, NT], F32, tag=f"r2{s}",
                              name=f"r2{s}_{p}") for s in AB]
        d["r"] = [r_pool.tile([128, NT], F32, tag=f"r{s}", name=f"r{s}_{p}")
                  for s in AB]
        d["ri"] = [r_pool.tile([128, NT], F32, tag=f"ri{s}", name=f"ri{s}_{p}")
                   for s in AB]
        d["e"] = [[None] * NT, [None] * NT]
        d["o"] = None

    def qk_unit(p, n):
        d = st8[p]
        for s in AB:
            er = er_pool.tile([128, S], BF16, tag=f"er{s}", name=f"er{s}_{p}_{n}")
            d["e"][s][n] = er
            hd = n // 8
            for h in range(2):
                st = st_pool.tile([128, 1024], F32)
                for j in range(2):
                    q0 = h * 1024 + j * 512
                    blk = slice(j * 512, (j + 1) * 512)
                    last_mm = (h == hd and
                               j == ((n % 8) * 128) // 512 and diag_mode == "pe")
                    nc.tensor.matmul(
                        st[:, blk],
                        lhsT=d["k2"][64 * s:64 * s + 64, n * 128:(n + 1) * 128],
                        rhs=d["q2"][64 * s:64 * s + 64, q0:q0 + 512],
                        start=True,
                        stop=not last_mm,
                        skip_group_check=True,
                    )
                    if last_mm:
                        c = (n % 8) * 128 - j * 512
                        nc.tensor.matmul(
                            st[:, j * 512 + c:j * 512 + c + 128],
                            lhsT=id_bf[:],
                            rhs=rg_bf[:],
                            start=False,
                            stop=True,
                            skip_group_check=True,
                        )
                if h == hd and diag_mode == "dve":
                    c = (n % 8) * 128
                    nc.vector.tensor_add(
                        st[:, c:c + 128], st[:, c:c + 128], rg_eye[:])
                nc.scalar.activation(
                    er[:, h * 1024:(h + 1) * 1024],
                    st[:],
                    mybir.ActivationFunctionType.Exp,
                    accum_out=d["r2"][s][:, h:h + 1, n:n + 1],
                )
            nc.vector.tensor_add(
                d["r"][s][:, n:n + 1], d["r2"][s][:, 0, n:n + 1],
                d["r2"][s][:, 1, n:n + 1])
            nc.vector.reciprocal_approx_fast(
                d["ri"][s][:, n:n + 1], d["r"][s][:, n:n + 1])

    def av_unit(p, m):
        d = st8[p]
        if d["o"] is None:
            d["o"] = o_pool.tile([128, S], F32, tag="o", name=f"o_{p}")
        for s in AB:
            vsc = vsc_pool.tile([128, 64], BF16, tag=f"vsc{s}", name=f"vsc{s}_{p}")
            nc.vector.tensor_scalar_mul(
                vsc[:], d["v"][s][:, m * 64:(m + 1) * 64], d["ri"][s][:, m:m + 1])
            for ch in range(4):
                cs = slice(ch * 512, (ch + 1) * 512)
                nc.tensor.matmul(
                    d["o"][64 * s:64 * s + 64, cs],
                    lhsT=vsc[:],
                    rhs=d["e"][s][m][:, cs],
                    start=(m == 0),
                    stop=(m == NT - 1),
                    skip_group_check=True,
                )

    def pair_end(p):
        d = st8[p]
        out_sb = osb_pool.tile([128, S], F16, tag="osb", name=f"osb_{p}")
        for h in range(4):
            cs = slice(h * 512, (h + 1) * 512)
            nc.vector.tensor_copy(out_sb[:, cs], d["o"][:, cs])
        nc.gpsimd.dma_start(ot[d["bh"][0]], out_sb[0:64, :])
        nc.sync.dma_start(ot[d["bh"][1]], out_sb[64:128, :])

    # flat software pipeline: global QK stream with AV trailing by av_lag,
    # flowing across pair boundaries
    total = NP * NT
    for g in range(total + av_lag):
        if g < total:
            p, n = divmod(g, NT)
            if n == 0:
                pair_start(p)
            qk_unit(p, n)
        if g >= av_lag:
            pm, mm = divmod(g - av_lag, NT)
            av_unit(pm, mm)
            if mm == NT - 1:
                pair_end(pm)


_NC_CACHE = {}


def build_nc(repeats=1, **cfg):
    key = (repeats, tuple(sorted(cfg.items())))
    if key in _NC_CACHE:
        return _NC_CACHE[key]
    nc = bacc.Bacc("TRN2", target_bir_lowering=False, debug=False)
    qt = nc.dram_tensor("qt", [NP, 2 * D, S], F16, kind="ExternalInput").ap()
    kt = nc.dram_tensor("kt", [NP, 2 * D, S], F16, kind="ExternalInput").ap()
    vs = nc.dram_tensor("vs", [BH_PER_CORE, 128, NT * 64], BF16, kind="ExternalInput").ap()
    rg = nc.dram_tensor("rg", [128, 128], F32, kind="ExternalInput").ap()
    rgb = nc.dram_tensor("rgb", [128, 256], BF16, kind="ExternalInput").ap()
    ot = nc.dram_tensor("ot", [BH_PER_CORE, D, S], F16, kind="ExternalOutput").ap()
    with tile.TileContext(nc) as tc, ExitStack() as ctx:
        if repeats == 1:
            _build_kernel(nc, tc, ctx, qt, kt, vs, rg, rgb, ot, cfg)
        else:
            # benchmarking mode: repeat the whole kernel body in an on-device
            # loop so per-iteration time can be extracted from wall clock
            with tc.For_i(0, repeats, 1,
                          staggered_reset=cfg.get("stag", False),
                          hint_engines=(mybir.EngineType.PE,
                                        mybir.EngineType.Activation,
                                        mybir.EngineType.DVE)):
                _build_kernel(nc, tc, ctx, qt, kt, vs, rg, rgb, ot, cfg)
    nc.compile()
    _NC_CACHE[key] = nc
    return nc


def _prep_inputs(qry, key, val, reg):
    """Host-side shard + layout prep. Returns per-core input maps."""
    q = np.ascontiguousarray(np.asarray(qry, dtype=np.float32)).reshape(BH, S, D)
    k = np.ascontiguousarray(np.asarray(key, dtype=np.float32)).reshape(BH, S, D)
    v = np.ascontiguousarray(np.asarray(val, dtype=np.float32)).reshape(BH, S, D)
    rg = (np.eye(128, dtype=np.float32) * np.float32(np.asarray(reg)))
    rgb = np.concatenate([np.eye(128, dtype=np.float32), rg],
                         axis=1).astype(ml_dtypes.bfloat16)

    in_maps = []
    for c in range(N_CORES):
        sl = slice(c * BH_PER_CORE, (c + 1) * BH_PER_CORE)
        qt = np.ascontiguousarray(
            q[sl].transpose(0, 2, 1).reshape(NP, 2 * D, S)
        ).astype(np.float16)                                          # [2, 128, S]
        kt = np.ascontiguousarray(
            k[sl].transpose(0, 2, 1).reshape(NP, 2 * D, S)
        ).astype(np.float16)                                          # [2, 128, S]
        vv = v[sl].reshape(BH_PER_CORE, NT, 128, D)
        vs = np.ascontiguousarray(vv.transpose(0, 2, 1, 3)).reshape(
            BH_PER_CORE, 128, NT * D).astype(ml_dtypes.bfloat16)      # [4, 128, 1024]
        in_maps.append({"qt": qt, "kt": kt, "vs": vs, "rg": rg,
                        "rgb": rgb})
    return in_maps


def kernel(qry, key, val, reg):
    nc = build_nc()
    in_maps = _prep_inputs(qry, key, val, reg)
    res = run_bass_kernel_spmd(nc, in_maps, list(range(N_CORES)))
    out = np.empty((BH, S, D), dtype=np.float32)
    for c in range(N_CORES):
        ot = res.results[c]["ot"].astype(np.float32)                 # [4, 64, S]
        for i in range(BH_PER_CORE):
            out[c * BH_PER_CORE + i] = ot[i].T
    return out.reshape(B, H, S, D)


# revision 24
# speedup vs baseline: 1.3089x; 1.0228x over previous
"""Trainium2 Bass kernel for nn_FAttention1d (attention with softmax over the
QUERY axis).

Reference computation (B=2, H=16, S=2048, D=64, fp32):
    att[b,h,q,k] = sum_d qry[b,h,q,d] * key[b,h,k,d]
    att += reg * I_S                      (diagonal in (q,k))
    att = softmax(att, axis=q)            (normalize over the QUERY axis)
    out[b,h,q,v] = sum_k att[b,h,q,k] * val[b,h,k,v]

Sharding: the 32 (b,h) pairs are split 4-per-core across 8 NeuronCores; the
whole S=2048 attention chain is local to a core.

Device-side layout: compute S^T = K @ Q^T with k on the partition axis, so the
q-axis softmax denominator is a free-axis reduction, and exp(S^T) tiles feed
the A^T V matmul directly as the moving operand:
    out^T[v,q] = sum_k (val[k,v] / r[k])_stationary @ exp(S^T)[k,q]
with r[k] = sum_q exp(S^T[k,q]) taken from the exp pass's ACT accumulator.

Engine split (per core, ACT is the roofline at ~183us busy):
  PE : QK^T 512-col f16 matmuls into [128,1024] PSUM tiles; reg*I diagonal
       injected as an extra 128-col matmul accumulation (identity stationary,
       rg_eye moving); AV bf16 matmuls interleaved at a lag, flowing across
       pair boundaries (flat software pipeline, no per-pair barrier).
  ACT: exp straight from PSUM -> SBUF bf16 with accum_out (the only r source
       that is cheap on HW; DVE accumulator reads measured ~1us each).
  DVE: r-half adds, reciprocal, bf16 val-row scaling, PSUM->SBUF out copy.
"""

import numpy as np
import ml_dtypes
from contextlib import ExitStack

import concourse.bass as bass
import concourse.mybir as mybir
import concourse.tile as tile
from concourse import bacc
from concourse.bass_utils import run_bass_kernel_spmd

B, H, S, D = 2, 16, 2048, 64
N_CORES = 8
BH = B * H                     # 32
BH_PER_CORE = BH // N_CORES    # 4
NP = BH_PER_CORE // 2          # 2 pairs per core
NT = S // 128                  # 16 k-tiles of 128
F32 = mybir.dt.float32
F16 = mybir.dt.float16
BF16 = mybir.dt.bfloat16
AB = (0, 1)


def _build_kernel(nc, tc, ctx, qt, kt, vs, rg, rgb, ot, cfg=None):
    cfg = cfg or {}
    av_lag = cfg.get("av_lag", 2)
    diag_mode = cfg.get("diag_mode", "dve")  # dve | pe
    skip_av = cfg.get("skip_av", False)      # timing probe: no AV/out
    skip_exp = cfg.get("skip_exp", False)    # timing probe: QK only
    no_vsc = cfg.get("no_vsc", False)        # timing probe: AV reads raw v
    no_out = cfg.get("no_out", False)        # timing probe: skip out copy/DMA
    av_noacc = cfg.get("av_noacc", False)    # timing probe: AV without accumulation
    const_pool = ctx.enter_context(tc.tile_pool(name="const", bufs=1))
    q_pool = ctx.enter_context(tc.tile_pool(name="q", bufs=2))
    k_pool = ctx.enter_context(tc.tile_pool(name="k", bufs=2))
    v_pool = ctx.enter_context(tc.tile_pool(name="v", bufs=2))
    er_pool = ctx.enter_context(tc.tile_pool(name="er",
                                             bufs=cfg.get("er_bufs", 4)))
    r_pool = ctx.enter_context(tc.tile_pool(name="r", bufs=2))
    vsc_pool = ctx.enter_context(tc.tile_pool(name="vsc", bufs=4))
    osb_pool = ctx.enter_context(tc.tile_pool(name="osb", bufs=2))
    st_pool = ctx.enter_context(tc.tile_pool(name="st", bufs=2, space="PSUM"))
    o_pool = ctx.enter_context(tc.tile_pool(name="o", bufs=1, space="PSUM"))

    rg_eye = const_pool.tile([128, 128], F32)
    nc.scalar.dma_start(rg_eye[:], rg[:])
    # bf16 [identity | reg*identity] for the PE diagonal injection:
    # st += id_bf^T @ rg_bf
    eyes = const_pool.tile([128, 256], BF16)
    nc.scalar.dma_start(eyes[:], rgb[:])
    id_bf = eyes[:, 0:128]
    rg_bf = eyes[:, 128:256]

    # per-pair state
    st8 = [dict() for _ in range(NP)]

    def pair_start(p):
        d = st8[p]
        bh = (2 * p, 2 * p + 1)
        d["bh"] = bh
        q2 = q_pool.tile([128, S], F16, tag="q2", name=f"q2_{p}")
        k2 = k_pool.tile([128, S], F16, tag="k2", name=f"k2_{p}")
        nc.sync.dma_start(q2[:, 0:512], qt[p][:, 0:512])
        nc.gpsimd.dma_start(k2[:, 0:128], kt[p][:, 0:128])
        nc.sync.dma_start(q2[:, 512:], qt[p][:, 512:])
        nc.gpsimd.dma_start(k2[:, 128:], kt[p][:, 128:])
        d["q2"], d["k2"] = q2, k2
        d["v"] = []
        for s in AB:
            v = v_pool.tile([128, NT * 64], BF16, tag=f"v{s}", name=f"v{s}_{p}")
            nc.gpsimd.dma_start(v[:], vs[bh[s]])
            d["v"].append(v)
        d["r2"] = [r_pool.tile([128, 2, NT], F32, tag=f"r2{s}",
                              name=f"r2{s}_{p}") for s in AB]
        d["r"] = [r_pool.tile([128, NT], F32, tag=f"r{s}", name=f"r{s}_{p}")
                  for s in AB]
        d["ri"] = [r_pool.tile([128, NT], F32, tag=f"ri{s}", name=f"ri{s}_{p}")
                   for s in AB]
        d["e"] = [[None] * NT, [None] * NT]
        d["o"] = None

    def qk_half(p, n, s, h):
        # one [128,1024] st fill + its exp; diag (reg*I) injected FIRST as a
        # start=True matmul the K matmul then accumulates onto (saves a K
        # reload vs diag-in-the-middle)
        d = st8[p]
        er = d["e"][s][n]
        hd = n // 8
        st = st_pool.tile([128, 1024], F32)
        dj = ((n % 8) * 128) // 512 if h == hd and diag_mode == "pe" else -1
        for j in range(2):
            q0 = h * 1024 + j * 512
            blk = slice(j * 512, (j + 1) * 512)
            nc.tensor.matmul(
                st[:, blk],
                lhsT=d["k2"][64 * s:64 * s + 64, n * 128:(n + 1) * 128],
                rhs=d["q2"][64 * s:64 * s + 64, q0:q0 + 512],
                start=True,
                stop=(j != dj),
                skip_group_check=True,
            )
            if j == dj:
                c = (n % 8) * 128 - dj * 512
                nc.tensor.matmul(
                    st[:, dj * 512 + c:dj * 512 + c + 128],
                    lhsT=id_bf[:],
                    rhs=rg_bf[:],
                    start=False,
                    stop=True,
                    skip_group_check=True,
                )
        if h == hd and diag_mode == "dve":
            c = (n % 8) * 128
            nc.vector.tensor_add(
                st[:, c:c + 128], st[:, c:c + 128], rg_eye[:])
        if not skip_exp:
            nc.scalar.activation(
                er[:, h * 1024:(h + 1) * 1024],
                st[:],
                mybir.ActivationFunctionType.Exp,
                accum_out=d["r2"][s][:, h:h + 1, n:n + 1],
            )


    def r_chain(p, n):
        d = st8[p]
        for s in AB:
            nc.vector.tensor_add(
                d["r"][s][:, n:n + 1], d["r2"][s][:, 0, n:n + 1],
                d["r2"][s][:, 1, n:n + 1])
            nc.vector.reciprocal_approx_fast(
                d["ri"][s][:, n:n + 1], d["r"][s][:, n:n + 1])

    def av_vsc(p, m):
        # dep-free by emission time: recip(m) was emitted a step earlier
        d = st8[p]
        if d["o"] is None:
            d["o"] = o_pool.tile([128, S], F32, tag="o", name=f"o_{p}")
        out = []
        for s in AB:
            if no_vsc:
                vsc = d["v"][s][:, m * 64:(m + 1) * 64]
            else:
                vsc = vsc_pool.tile([128, 64], BF16, tag=f"vsc{s}",
                                    name=f"vsc{s}_{p}")
                nc.vector.tensor_scalar_mul(
                    vsc[:], d["v"][s][:, m * 64:(m + 1) * 64],
                    d["ri"][s][:, m:m + 1])
            out.append(vsc)
        return out

    def av_quarter(p, m, vscs, s, chs):
        # 2 of the 8 AV matmuls for k-tile m: head s, chunk pair chs
        d = st8[p]
        for ch in chs:
            cs = slice(ch * 512, (ch + 1) * 512)
            nc.tensor.matmul(
                d["o"][64 * s:64 * s + 64, cs],
                lhsT=vscs[s][:],
                rhs=d["e"][s][m][:, cs],
                start=True if av_noacc else (m == 0),
                stop=True if av_noacc else (m == NT - 1),
                skip_group_check=True,
            )

    def pair_end(p):
        if no_out:
            return
        d = st8[p]
        out_sb = osb_pool.tile([128, S], F16, tag="osb", name=f"osb_{p}")
        for h in range(4):
            cs = slice(h * 512, (h + 1) * 512)
            nc.vector.tensor_copy(out_sb[:, cs], d["o"][:, cs])
        nc.gpsimd.dma_start(ot[d["bh"][0]], out_sb[0:64, :])
        nc.sync.dma_start(ot[d["bh"][1]], out_sb[64:128, :])

    # flat software pipeline: global QK stream with AV trailing by av_lag,
    # flowing across pair boundaries
    total = NP * NT
    for g in range(total + av_lag):
        if g >= 1 and g - 1 < total and not skip_exp:
            pr, nr = divmod(g - 1, NT)
            r_chain(pr, nr)
        do_av = g >= av_lag and not (skip_av or skip_exp)
        if do_av:
            pm, mm = divmod(g - av_lag, NT)
            vscs = av_vsc(pm, mm)
        if g < total:
            p, n = divmod(g, NT)
            if n == 0:
                pair_start(p)
            for s in AB:
                er = er_pool.tile([128, S], BF16, tag=f"er{s}",
                                  name=f"er{s}_{p}_{n}")
                st8[p]["e"][s][n] = er
            if cfg.get("fine", True):
                # fine-grained interleave: 2 AV matmuls after each QK half so
                # the in-order PE stream never delays an st fill by ~>400ns
                for i, (s, h) in enumerate(((0, 0), (0, 1), (1, 0), (1, 1))):
                    qk_half(p, n, s, h)
                    if do_av:
                        av_quarter(pm, mm, vscs, i // 2, (2 * (i % 2),
                                                          2 * (i % 2) + 1))
            else:
                # coarse: QK for both heads, then the full AV unit
                for s, h in ((0, 0), (0, 1), (1, 0), (1, 1)):
                    qk_half(p, n, s, h)
                if do_av:
                    for s in AB:
                        av_quarter(pm, mm, vscs, s, (0, 1))
                        av_quarter(pm, mm, vscs, s, (2, 3))
        elif do_av:
            for s in AB:
                av_quarter(pm, mm, vscs, s, (0, 1))
                av_quarter(pm, mm, vscs, s, (2, 3))
        if do_av and mm == NT - 1:
            pair_end(pm)


_NC_CACHE = {}


def build_nc(repeats=1, **cfg):
    key = (repeats, tuple(sorted(cfg.items())))
    if key in _NC_CACHE:
        return _NC_CACHE[key]
    nc = bacc.Bacc("TRN2", target_bir_lowering=False, debug=False)
    qt = nc.dram_tensor("qt", [NP, 2 * D, S], F16, kind="ExternalInput").ap()
    kt = nc.dram_tensor("kt", [NP, 2 * D, S], F16, kind="ExternalInput").ap()
    vs = nc.dram_tensor("vs", [BH_PER_CORE, 128, NT * 64], BF16, kind="ExternalInput").ap()
    rg = nc.dram_tensor("rg", [128, 128], F32, kind="ExternalInput").ap()
    rgb = nc.dram_tensor("rgb", [128, 256], BF16, kind="ExternalInput").ap()
    ot = nc.dram_tensor("ot", [BH_PER_CORE, D, S], F16, kind="ExternalOutput").ap()
    with tile.TileContext(nc) as tc, ExitStack() as ctx:
        if repeats == 1:
            _build_kernel(nc, tc, ctx, qt, kt, vs, rg, rgb, ot, cfg)
        else:
            # benchmarking mode: repeat the whole kernel body in an on-device
            # loop so per-iteration time can be extracted from wall clock
            with tc.For_i(0, repeats, 1,
                          staggered_reset=cfg.get("stag", False),
                          hint_engines=(mybir.EngineType.PE,
                                        mybir.EngineType.Activation,
                                        mybir.EngineType.DVE)):
                _build_kernel(nc, tc, ctx, qt, kt, vs, rg, rgb, ot, cfg)
    nc.compile()
    _NC_CACHE[key] = nc
    return nc


def _prep_inputs(qry, key, val, reg):
    """Host-side shard + layout prep. Returns per-core input maps."""
    q = np.ascontiguousarray(np.asarray(qry, dtype=np.float32)).reshape(BH, S, D)
    k = np.ascontiguousarray(np.asarray(key, dtype=np.float32)).reshape(BH, S, D)
    v = np.ascontiguousarray(np.asarray(val, dtype=np.float32)).reshape(BH, S, D)
    rg = (np.eye(128, dtype=np.float32) * np.float32(np.asarray(reg)))
    rgb = np.concatenate([np.eye(128, dtype=np.float32), rg],
                         axis=1).astype(ml_dtypes.bfloat16)

    in_maps = []
    for c in range(N_CORES):
        sl = slice(c * BH_PER_CORE, (c + 1) * BH_PER_CORE)
        qt = np.ascontiguousarray(
            q[sl].transpose(0, 2, 1).reshape(NP, 2 * D, S)
        ).astype(np.float16)                                          # [2, 128, S]
        kt = np.ascontiguousarray(
            k[sl].transpose(0, 2, 1).reshape(NP, 2 * D, S)
        ).astype(np.float16)                                          # [2, 128, S]
        vv = v[sl].reshape(BH_PER_CORE, NT, 128, D)
        vs = np.ascontiguousarray(vv.transpose(0, 2, 1, 3)).reshape(
            BH_PER_CORE, 128, NT * D).astype(ml_dtypes.bfloat16)      # [4, 128, 1024]
        in_maps.append({"qt": qt, "kt": kt, "vs": vs, "rg": rg,
                        "rgb": rgb})
    return in_maps


def kernel(qry, key, val, reg):
    nc = build_nc()
    in_maps = _prep_inputs(qry, key, val, reg)
    res = run_bass_kernel_spmd(nc, in_maps, list(range(N_CORES)))
    out = np.empty((BH, S, D), dtype=np.float32)
    for c in range(N_CORES):
        ot = res.results[c]["ot"].astype(np.float32)                 # [4, 64, S]
        for i in range(BH_PER_CORE):
            out[c * BH_PER_CORE + i] = ot[i].T
    return out.reshape(B, H, S, D)


# revision 26
# speedup vs baseline: 1.8317x; 1.3994x over previous
"""Trainium2 Bass kernel for nn_FAttention1d (attention with softmax over the
QUERY axis).

Reference computation (B=2, H=16, S=2048, D=64, fp32):
    att[b,h,q,k] = sum_d qry[b,h,q,d] * key[b,h,k,d]
    att += reg * I_S                      (diagonal in (q,k))
    att = softmax(att, axis=q)            (normalize over the QUERY axis)
    out[b,h,q,v] = sum_k att[b,h,q,k] * val[b,h,k,v]

Sharding: the 32 (b,h) pairs are split 4-per-core across 8 NeuronCores; the
whole S=2048 attention chain is local to a core.

Device-side layout: compute S^T = K @ Q^T with k on the partition axis, so the
q-axis softmax denominator is a free-axis reduction, and exp(S^T) tiles feed
the A^T V matmul directly as the moving operand:
    out^T[v,q] = sum_k (val[k,v] / r[k])_stationary @ exp(S^T)[k,q]
with r[k] = sum_q exp(S^T[k,q]) taken from the exp pass's ACT accumulator.

Engine split (per core, ACT is the roofline at ~183us busy):
  PE : QK^T 512-col f16 matmuls into [128,1024] PSUM tiles with the
       contraction PADDED from 64 to 128 rows (zero rows 64-127) so QK and AV
       stationaries share one PE row configuration -- switching 64<->128
       contraction between interleaved QK/AV cost ~55us of PE stalls on HW;
       AV bf16 matmuls interleaved at a lag in a flat cross-pair pipeline.
  ACT: exp straight from PSUM -> SBUF bf16 with accum_out (the only r source
       that is cheap on HW; DVE accumulator reads measured ~1us each).
  DVE: reg*I diag adds into PSUM, r-half adds, reciprocal, bf16 val-row
       scaling, PSUM->SBUF out copy (fp16 staging, dual-queue DMA).
"""

import numpy as np
import ml_dtypes
from contextlib import ExitStack

import concourse.bass as bass
import concourse.mybir as mybir
import concourse.tile as tile
from concourse import bacc
from concourse.bass_utils import run_bass_kernel_spmd

B, H, S, D = 2, 16, 2048, 64
N_CORES = 8
BH = B * H                     # 32
BH_PER_CORE = BH // N_CORES    # 4
NP = BH_PER_CORE // 2          # 2 pairs per core
NT = S // 128                  # 16 k-tiles of 128
F32 = mybir.dt.float32
F16 = mybir.dt.float16
BF16 = mybir.dt.bfloat16
AB = (0, 1)


def _build_kernel(nc, tc, ctx, qt, kt, vs, rg, rgb, ot, cfg=None):
    cfg = cfg or {}
    av_lag = cfg.get("av_lag", 2)
    diag_mode = cfg.get("diag_mode", "dve")  # dve | pe
    skip_av = cfg.get("skip_av", False)      # timing probe: no AV/out
    skip_exp = cfg.get("skip_exp", False)    # timing probe: QK only
    no_vsc = cfg.get("no_vsc", False)        # timing probe: AV reads raw v
    no_out = cfg.get("no_out", False)        # timing probe: skip out copy/DMA
    av_noacc = cfg.get("av_noacc", False)    # timing probe: AV without accumulation
    cpad = cfg.get("cpad", True)             # pad QK contraction 64->128
    const_pool = ctx.enter_context(tc.tile_pool(name="const", bufs=1))
    q_pool = ctx.enter_context(tc.tile_pool(name="q", bufs=2))
    k_pool = ctx.enter_context(tc.tile_pool(name="k", bufs=2))
    v_pool = ctx.enter_context(tc.tile_pool(name="v", bufs=2))
    er_pool = ctx.enter_context(tc.tile_pool(name="er",
                                             bufs=cfg.get("er_bufs", 4)))
    r_pool = ctx.enter_context(tc.tile_pool(name="r", bufs=2))
    vsc_pool = ctx.enter_context(tc.tile_pool(name="vsc", bufs=4))
    osb_pool = ctx.enter_context(tc.tile_pool(name="osb", bufs=2))
    st_pool = ctx.enter_context(tc.tile_pool(name="st", bufs=2, space="PSUM"))
    o_pool = ctx.enter_context(tc.tile_pool(name="o", bufs=1, space="PSUM"))

    rg_eye = const_pool.tile([128, 128], F32)
    nc.scalar.dma_start(rg_eye[:], rg[:])
    # bf16 [identity | reg*identity] for the PE diagonal injection:
    # st += id_bf^T @ rg_bf
    eyes = const_pool.tile([128, 256], BF16)
    nc.scalar.dma_start(eyes[:], rgb[:])
    id_bf = eyes[:, 0:128]
    rg_bf = eyes[:, 128:256]

    qp = kp = None
    if cpad:
        # [s][parity] persistent tiles; rows 64-127 stay zero
        qp = [[const_pool.tile([128, S], F16, name=f"qp{s}_{i}") for i in (0, 1)]
              for s in AB]
        kp = [[const_pool.tile([128, S], F16, name=f"kp{s}_{i}") for i in (0, 1)]
              for s in AB]
        for s in AB:
            for i in (0, 1):
                nc.gpsimd.memset(qp[s][i][64:128, :], 0.0)
                nc.gpsimd.memset(kp[s][i][64:128, :], 0.0)

    # per-pair state
    st8 = [dict() for _ in range(NP)]

    def pair_start(p):
        d = st8[p]
        bh = (2 * p, 2 * p + 1)
        d["bh"] = bh
        if cpad:
            i = p % 2
            for s in AB:
                nc.sync.dma_start(qp[s][i][0:64, 0:512], qt[p][64 * s:64 * s + 64, 0:512])
                nc.gpsimd.dma_start(kp[s][i][0:64, 0:128], kt[p][64 * s:64 * s + 64, 0:128])
                nc.sync.dma_start(qp[s][i][0:64, 512:], qt[p][64 * s:64 * s + 64, 512:])
                nc.gpsimd.dma_start(kp[s][i][0:64, 128:], kt[p][64 * s:64 * s + 64, 128:])
            d["qp"] = [qp[s][i] for s in AB]
            d["kp"] = [kp[s][i] for s in AB]
        else:
            q2 = q_pool.tile([128, S], F16, tag="q2", name=f"q2_{p}")
            k2 = k_pool.tile([128, S], F16, tag="k2", name=f"k2_{p}")
            nc.sync.dma_start(q2[:, 0:512], qt[p][:, 0:512])
            nc.gpsimd.dma_start(k2[:, 0:128], kt[p][:, 0:128])
            nc.sync.dma_start(q2[:, 512:], qt[p][:, 512:])
            nc.gpsimd.dma_start(k2[:, 128:], kt[p][:, 128:])
            d["q2"], d["k2"] = q2, k2
        d["v"] = []
        for s in AB:
            v = v_pool.tile([128, NT * 64], BF16, tag=f"v{s}", name=f"v{s}_{p}")
            nc.gpsimd.dma_start(v[:], vs[bh[s]])
            d["v"].append(v)
        d["r2"] = [r_pool.tile([128, 2, NT], F32, tag=f"r2{s}",
                              name=f"r2{s}_{p}") for s in AB]
        d["r"] = [r_pool.tile([128, NT], F32, tag=f"r{s}", name=f"r{s}_{p}")
                  for s in AB]
        d["ri"] = [r_pool.tile([128, NT], F32, tag=f"ri{s}", name=f"ri{s}_{p}")
                   for s in AB]
        d["e"] = [[None] * NT, [None] * NT]
        d["o"] = None

    def qk_half(p, n, s, h):
        # one [128,1024] st fill + its exp; diag (reg*I) injected FIRST as a
        # start=True matmul the K matmul then accumulates onto (saves a K
        # reload vs diag-in-the-middle)
        d = st8[p]
        er = d["e"][s][n]
        hd = n // 8
        st = st_pool.tile([128, 1024], F32)
        dj = ((n % 8) * 128) // 512 if h == hd and diag_mode == "pe" else -1
        for j in range(2):
            q0 = h * 1024 + j * 512
            blk = slice(j * 512, (j + 1) * 512)
            if cpad:
                lhsT = d["kp"][s][:, n * 128:(n + 1) * 128]
                rhs = d["qp"][s][:, q0:q0 + 512]
            else:
                lhsT = d["k2"][64 * s:64 * s + 64, n * 128:(n + 1) * 128]
                rhs = d["q2"][64 * s:64 * s + 64, q0:q0 + 512]
            nc.tensor.matmul(
                st[:, blk],
                lhsT=lhsT,
                rhs=rhs,
                start=True,
                stop=(j != dj),
                skip_group_check=True,
            )
            if j == dj:
                c = (n % 8) * 128 - dj * 512
                nc.tensor.matmul(
                    st[:, dj * 512 + c:dj * 512 + c + 128],
                    lhsT=id_bf[:],
                    rhs=rg_bf[:],
                    start=False,
                    stop=True,
                    skip_group_check=True,
                )
        if h == hd and diag_mode == "dve":
            c = (n % 8) * 128
            nc.vector.tensor_add(
                st[:, c:c + 128], st[:, c:c + 128], rg_eye[:])
        if not skip_exp:
            nc.scalar.activation(
                er[:, h * 1024:(h + 1) * 1024],
                st[:],
                mybir.ActivationFunctionType.Exp,
                accum_out=d["r2"][s][:, h:h + 1, n:n + 1],
            )


    def r_chain(p, n):
        d = st8[p]
        for s in AB:
            nc.vector.tensor_add(
                d["r"][s][:, n:n + 1], d["r2"][s][:, 0, n:n + 1],
                d["r2"][s][:, 1, n:n + 1])
            nc.vector.reciprocal_approx_fast(
                d["ri"][s][:, n:n + 1], d["r"][s][:, n:n + 1])

    def av_vsc(p, m):
        # dep-free by emission time: recip(m) was emitted a step earlier
        d = st8[p]
        if d["o"] is None:
            d["o"] = o_pool.tile([128, S], F32, tag="o", name=f"o_{p}")
        out = []
        for s in AB:
            if no_vsc:
                vsc = d["v"][s][:, m * 64:(m + 1) * 64]
            else:
                vsc = vsc_pool.tile([128, 64], BF16, tag=f"vsc{s}",
                                    name=f"vsc{s}_{p}")
                nc.vector.tensor_scalar_mul(
                    vsc[:], d["v"][s][:, m * 64:(m + 1) * 64],
                    d["ri"][s][:, m:m + 1])
            out.append(vsc)
        return out

    def av_quarter(p, m, vscs, s, chs):
        # 2 of the 8 AV matmuls for k-tile m: head s, chunk pair chs
        d = st8[p]
        for ch in chs:
            cs = slice(ch * 512, (ch + 1) * 512)
            nc.tensor.matmul(
                d["o"][64 * s:64 * s + 64, cs],
                lhsT=vscs[s][:],
                rhs=d["e"][s][m][:, cs],
                start=True if av_noacc else (m == 0),
                stop=True if av_noacc else (m == NT - 1),
                skip_group_check=True,
            )

    def pair_end(p):
        if no_out:
            return
        d = st8[p]
        out_sb = osb_pool.tile([128, S], F16, tag="osb", name=f"osb_{p}")
        for h in range(4):
            cs = slice(h * 512, (h + 1) * 512)
            nc.vector.tensor_copy(out_sb[:, cs], d["o"][:, cs])
        nc.gpsimd.dma_start(ot[d["bh"][0]], out_sb[0:64, :])
        nc.sync.dma_start(ot[d["bh"][1]], out_sb[64:128, :])

    # flat software pipeline: global QK stream with AV trailing by av_lag,
    # flowing across pair boundaries
    total = NP * NT
    for g in range(total + av_lag):
        if g >= 1 and g - 1 < total and not skip_exp:
            pr, nr = divmod(g - 1, NT)
            r_chain(pr, nr)
        do_av = g >= av_lag and not (skip_av or skip_exp)
        if do_av:
            pm, mm = divmod(g - av_lag, NT)
            vscs = av_vsc(pm, mm)
        if g < total:
            p, n = divmod(g, NT)
            if n == 0:
                pair_start(p)
            for s in AB:
                er = er_pool.tile([128, S], BF16, tag=f"er{s}",
                                  name=f"er{s}_{p}_{n}")
                st8[p]["e"][s][n] = er
            if cfg.get("fine", True):
                # fine-grained interleave: 2 AV matmuls after each QK half so
                # the in-order PE stream never delays an st fill by ~>400ns
                for i, (s, h) in enumerate(((0, 0), (0, 1), (1, 0), (1, 1))):
                    qk_half(p, n, s, h)
                    if do_av:
                        av_quarter(pm, mm, vscs, i // 2, (2 * (i % 2),
                                                          2 * (i % 2) + 1))
            else:
                # coarse: QK for both heads, then the full AV unit
                for s, h in ((0, 0), (0, 1), (1, 0), (1, 1)):
                    qk_half(p, n, s, h)
                if do_av:
                    for s in AB:
                        av_quarter(pm, mm, vscs, s, (0, 1))
                        av_quarter(pm, mm, vscs, s, (2, 3))
        elif do_av:
            for s in AB:
                av_quarter(pm, mm, vscs, s, (0, 1))
                av_quarter(pm, mm, vscs, s, (2, 3))
        if do_av and mm == NT - 1:
            pair_end(pm)


_NC_CACHE = {}


def build_nc(repeats=1, **cfg):
    key = (repeats, tuple(sorted(cfg.items())))
    if key in _NC_CACHE:
        return _NC_CACHE[key]
    nc = bacc.Bacc("TRN2", target_bir_lowering=False, debug=False)
    qt = nc.dram_tensor("qt", [NP, 2 * D, S], F16, kind="ExternalInput").ap()
    kt = nc.dram_tensor("kt", [NP, 2 * D, S], F16, kind="ExternalInput").ap()
    vs = nc.dram_tensor("vs", [BH_PER_CORE, 128, NT * 64], BF16, kind="ExternalInput").ap()
    rg = nc.dram_tensor("rg", [128, 128], F32, kind="ExternalInput").ap()
    rgb = nc.dram_tensor("rgb", [128, 256], BF16, kind="ExternalInput").ap()
    ot = nc.dram_tensor("ot", [BH_PER_CORE, D, S], F16, kind="ExternalOutput").ap()
    with tile.TileContext(nc) as tc, ExitStack() as ctx:
        if repeats == 1:
            _build_kernel(nc, tc, ctx, qt, kt, vs, rg, rgb, ot, cfg)
        else:
            # benchmarking mode: repeat the whole kernel body in an on-device
            # loop so per-iteration time can be extracted from wall clock
            with tc.For_i(0, repeats, 1,
                          staggered_reset=cfg.get("stag", True),
                          hint_engines=(mybir.EngineType.PE,
                                        mybir.EngineType.Activation,
                                        mybir.EngineType.DVE)):
                _build_kernel(nc, tc, ctx, qt, kt, vs, rg, rgb, ot, cfg)
    nc.compile()
    _NC_CACHE[key] = nc
    return nc


def _prep_inputs(qry, key, val, reg):
    """Host-side shard + layout prep. Returns per-core input maps."""
    q = np.ascontiguousarray(np.asarray(qry, dtype=np.float32)).reshape(BH, S, D)
    k = np.ascontiguousarray(np.asarray(key, dtype=np.float32)).reshape(BH, S, D)
    v = np.ascontiguousarray(np.asarray(val, dtype=np.float32)).reshape(BH, S, D)
    rg = (np.eye(128, dtype=np.float32) * np.float32(np.asarray(reg)))
    rgb = np.concatenate([np.eye(128, dtype=np.float32), rg],
                         axis=1).astype(ml_dtypes.bfloat16)

    in_maps = []
    for c in range(N_CORES):
        sl = slice(c * BH_PER_CORE, (c + 1) * BH_PER_CORE)
        qt = np.ascontiguousarray(
            q[sl].transpose(0, 2, 1).reshape(NP, 2 * D, S)
        ).astype(np.float16)                                          # [2, 128, S]
        kt = np.ascontiguousarray(
            k[sl].transpose(0, 2, 1).reshape(NP, 2 * D, S)
        ).astype(np.float16)                                          # [2, 128, S]
        vv = v[sl].reshape(BH_PER_CORE, NT, 128, D)
        vs = np.ascontiguousarray(vv.transpose(0, 2, 1, 3)).reshape(
            BH_PER_CORE, 128, NT * D).astype(ml_dtypes.bfloat16)      # [4, 128, 1024]
        in_maps.append({"qt": qt, "kt": kt, "vs": vs, "rg": rg,
                        "rgb": rgb})
    return in_maps


def kernel(qry, key, val, reg):
    nc = build_nc()
    in_maps = _prep_inputs(qry, key, val, reg)
    res = run_bass_kernel_spmd(nc, in_maps, list(range(N_CORES)))
    out = np.empty((BH, S, D), dtype=np.float32)
    for c in range(N_CORES):
        ot = res.results[c]["ot"].astype(np.float32)                 # [4, 64, S]
        for i in range(BH_PER_CORE):
            out[c * BH_PER_CORE + i] = ot[i].T
    return out.reshape(B, H, S, D)


# revision 28
# speedup vs baseline: 1.8377x; 1.0033x over previous
"""Trainium2 Bass kernel for nn_FAttention1d (attention with softmax over the
QUERY axis).

Reference computation (B=2, H=16, S=2048, D=64, fp32):
    att[b,h,q,k] = sum_d qry[b,h,q,d] * key[b,h,k,d]
    att += reg * I_S                      (diagonal in (q,k))
    att = softmax(att, axis=q)            (normalize over the QUERY axis)
    out[b,h,q,v] = sum_k att[b,h,q,k] * val[b,h,k,v]

Sharding: the 32 (b,h) pairs are split 4-per-core across 8 NeuronCores; the
whole S=2048 attention chain is local to a core.

Device-side layout: compute S^T = K @ Q^T with k on the partition axis, so the
q-axis softmax denominator is a free-axis reduction, and exp(S^T) tiles feed
the A^T V matmul directly as the moving operand:
    out^T[v,q] = sum_k (val[k,v] / r[k])_stationary @ exp(S^T)[k,q]
with r[k] = sum_q exp(S^T[k,q]) taken from the exp pass's ACT accumulator.

Engine split (per core, ACT is the roofline at ~183us busy):
  PE : QK^T 512-col f16 matmuls into [128,1024] PSUM tiles with the
       contraction PADDED from 64 to 128 rows (zero rows 64-127) so QK and AV
       stationaries share one PE row configuration -- switching 64<->128
       contraction between interleaved QK/AV cost ~55us of PE stalls on HW;
       AV bf16 matmuls interleaved at a lag in a flat cross-pair pipeline.
  ACT: exp straight from PSUM -> SBUF bf16 with accum_out (the only r source
       that is cheap on HW; DVE accumulator reads measured ~1us each).
  DVE: reg*I diag adds into PSUM, r-half adds, reciprocal, bf16 val-row
       scaling, PSUM->SBUF out copy (fp16 staging, dual-queue DMA).
"""

import numpy as np
import ml_dtypes
from contextlib import ExitStack

import concourse.bass as bass
import concourse.mybir as mybir
import concourse.tile as tile
from concourse import bacc
from concourse.bass_utils import run_bass_kernel_spmd

B, H, S, D = 2, 16, 2048, 64
N_CORES = 8
BH = B * H                     # 32
BH_PER_CORE = BH // N_CORES    # 4
NP = BH_PER_CORE // 2          # 2 pairs per core
NT = S // 128                  # 16 k-tiles of 128
F32 = mybir.dt.float32
F16 = mybir.dt.float16
BF16 = mybir.dt.bfloat16
AB = (0, 1)


def _build_kernel(nc, tc, ctx, qt, kt, vs, rg, rgb, ot, cfg=None):
    cfg = cfg or {}
    av_lag = cfg.get("av_lag", 2)
    diag_mode = cfg.get("diag_mode", "dve")  # dve | pe
    skip_av = cfg.get("skip_av", False)      # timing probe: no AV/out
    skip_exp = cfg.get("skip_exp", False)    # timing probe: QK only
    no_vsc = cfg.get("no_vsc", False)        # timing probe: AV reads raw v
    no_out = cfg.get("no_out", False)        # timing probe: skip out copy/DMA
    av_noacc = cfg.get("av_noacc", False)    # timing probe: AV without accumulation
    cpad = cfg.get("cpad", True)             # pad QK contraction 64->128
    r_mode = cfg.get("r_mode", "act")        # act | dve | split | probe | mix
    mixk = cfg.get("mixk", 1)                # mix: n%4<mixk tiles use ACT accum
    const_pool = ctx.enter_context(tc.tile_pool(name="const", bufs=1))
    q_pool = ctx.enter_context(tc.tile_pool(name="q", bufs=2))
    k_pool = ctx.enter_context(tc.tile_pool(name="k", bufs=2))
    v_pool = ctx.enter_context(tc.tile_pool(name="v", bufs=2))
    er_pool = ctx.enter_context(tc.tile_pool(name="er",
                                             bufs=cfg.get("er_bufs", 4)))
    r_pool = ctx.enter_context(tc.tile_pool(name="r", bufs=2))
    vsc_pool = ctx.enter_context(tc.tile_pool(name="vsc", bufs=4))
    osb_pool = ctx.enter_context(tc.tile_pool(name="osb", bufs=2))
    ef_pool = ctx.enter_context(tc.tile_pool(name="ef", bufs=4))
    st_pool = ctx.enter_context(tc.tile_pool(name="st", bufs=2, space="PSUM"))
    o_pool = ctx.enter_context(tc.tile_pool(name="o", bufs=1, space="PSUM"))

    rg_eye = const_pool.tile([128, 128], F32)
    nc.scalar.dma_start(rg_eye[:], rg[:])
    # bf16 [identity | reg*identity] for the PE diagonal injection:
    # st += id_bf^T @ rg_bf
    eyes = const_pool.tile([128, 256], BF16)
    nc.scalar.dma_start(eyes[:], rgb[:])
    id_bf = eyes[:, 0:128]
    rg_bf = eyes[:, 128:256]

    qp = kp = None
    if cpad:
        # [s][parity] persistent tiles; rows 64-127 stay zero
        qp = [[const_pool.tile([128, S], F16, name=f"qp{s}_{i}") for i in (0, 1)]
              for s in AB]
        kp = [[const_pool.tile([128, S], F16, name=f"kp{s}_{i}") for i in (0, 1)]
              for s in AB]
        for s in AB:
            for i in (0, 1):
                nc.gpsimd.memset(qp[s][i][64:128, :], 0.0)
                nc.gpsimd.memset(kp[s][i][64:128, :], 0.0)

    # per-pair state
    st8 = [dict() for _ in range(NP)]

    def pair_start(p):
        d = st8[p]
        bh = (2 * p, 2 * p + 1)
        d["bh"] = bh
        if cpad:
            i = p % 2
            for s in AB:
                nc.sync.dma_start(qp[s][i][0:64, 0:512], qt[p][64 * s:64 * s + 64, 0:512])
                nc.gpsimd.dma_start(kp[s][i][0:64, 0:128], kt[p][64 * s:64 * s + 64, 0:128])
                nc.sync.dma_start(qp[s][i][0:64, 512:], qt[p][64 * s:64 * s + 64, 512:])
                nc.gpsimd.dma_start(kp[s][i][0:64, 128:], kt[p][64 * s:64 * s + 64, 128:])
            d["qp"] = [qp[s][i] for s in AB]
            d["kp"] = [kp[s][i] for s in AB]
        else:
            q2 = q_pool.tile([128, S], F16, tag="q2", name=f"q2_{p}")
            k2 = k_pool.tile([128, S], F16, tag="k2", name=f"k2_{p}")
            nc.sync.dma_start(q2[:, 0:512], qt[p][:, 0:512])
            nc.gpsimd.dma_start(k2[:, 0:128], kt[p][:, 0:128])
            nc.sync.dma_start(q2[:, 512:], qt[p][:, 512:])
            nc.gpsimd.dma_start(k2[:, 128:], kt[p][:, 128:])
            d["q2"], d["k2"] = q2, k2
        d["v"] = []
        for s in AB:
            v = v_pool.tile([128, NT * 64], BF16, tag=f"v{s}", name=f"v{s}_{p}")
            nc.gpsimd.dma_start(v[:], vs[bh[s]])
            d["v"].append(v)
        d["r2"] = [r_pool.tile([128, 2, NT], F32, tag=f"r2{s}",
                              name=f"r2{s}_{p}") for s in AB]
        d["r"] = [r_pool.tile([128, NT], F32, tag=f"r{s}", name=f"r{s}_{p}")
                  for s in AB]
        d["ri"] = [r_pool.tile([128, NT], F32, tag=f"ri{s}", name=f"ri{s}_{p}")
                   for s in AB]
        d["e"] = [[None] * NT, [None] * NT]
        d["o"] = None

    def qk_half(p, n, s, h):
        # one [128,1024] st fill + its exp; diag (reg*I) injected FIRST as a
        # start=True matmul the K matmul then accumulates onto (saves a K
        # reload vs diag-in-the-middle)
        d = st8[p]
        er = d["e"][s][n]
        hd = n // 8
        st = st_pool.tile([128, 1024], F32)
        dj = ((n % 8) * 128) // 512 if h == hd and diag_mode == "pe" else -1
        for j in range(2):
            q0 = h * 1024 + j * 512
            blk = slice(j * 512, (j + 1) * 512)
            if cpad:
                lhsT = d["kp"][s][:, n * 128:(n + 1) * 128]
                rhs = d["qp"][s][:, q0:q0 + 512]
            else:
                lhsT = d["k2"][64 * s:64 * s + 64, n * 128:(n + 1) * 128]
                rhs = d["q2"][64 * s:64 * s + 64, q0:q0 + 512]
            nc.tensor.matmul(
                st[:, blk],
                lhsT=lhsT,
                rhs=rhs,
                start=True,
                stop=(j != dj),
                skip_group_check=True,
            )
            if j == dj:
                c = (n % 8) * 128 - dj * 512
                nc.tensor.matmul(
                    st[:, dj * 512 + c:dj * 512 + c + 128],
                    lhsT=id_bf[:],
                    rhs=rg_bf[:],
                    start=False,
                    stop=True,
                    skip_group_check=True,
                )
        if h == hd and diag_mode == "dve":
            c = (n % 8) * 128
            nc.vector.tensor_add(
                st[:, c:c + 128], st[:, c:c + 128], rg_eye[:])
        if not skip_exp:
            act_acc = (r_mode == "act" or (r_mode == "split" and n % 2 == 0)
                       or (r_mode == "mix" and n % 4 < mixk))
            if act_acc:
                nc.scalar.activation(
                    er[:, h * 1024:(h + 1) * 1024],
                    st[:],
                    mybir.ActivationFunctionType.Exp,
                    accum_out=d["r2"][s][:, h:h + 1, n:n + 1],
                )
            else:
                nc.scalar.activation(
                    er[:, h * 1024:(h + 1) * 1024],
                    st[:],
                    mybir.ActivationFunctionType.Exp,
                )


    def r_chain(p, n):
        d = st8[p]
        act_acc = (r_mode == "act" or (r_mode == "split" and n % 2 == 0)
                   or (r_mode == "mix" and n % 4 < mixk))
        for s in AB:
            if act_acc:
                nc.vector.tensor_add(
                    d["r"][s][:, n:n + 1], d["r2"][s][:, 0, n:n + 1],
                    d["r2"][s][:, 1, n:n + 1])
            elif r_mode == "mix":
                # r via a plain DVE free-axis reduce (no accumulator -- DVE
                # accum_out costs ~2us/instr on HW); AV reads er directly
                nc.vector.tensor_reduce(
                    d["r"][s][:, n:n + 1], d["e"][s][n][:],
                    axis=mybir.AxisListType.X, op=mybir.AluOpType.add)
            else:
                # r via a fast-mode DVE copy-accum over the whole exp tile;
                # AV then reads the copy ("probe" skips accum: timing only)
                er = d["e"][s][n]
                ef = ef_pool.tile([128, S], BF16, tag=f"ef{s}",
                                  name=f"ef{s}_{p}_{n}")
                d["e"][s][n] = ef
                if r_mode == "probe":
                    nc.vector.tensor_scalar(
                        ef[:], er[:], 1.0, 0.0,
                        op0=mybir.AluOpType.mult, op1=mybir.AluOpType.add)
                    if n == 0:
                        nc.vector.memset(d["r"][s][:, :], 1.0)
                else:
                    nc.vector.tensor_scalar(
                        ef[:], er[:], 1.0, 0.0,
                        op0=mybir.AluOpType.mult, op1=mybir.AluOpType.add,
                        accum_out=d["r"][s][:, n:n + 1])
            nc.vector.reciprocal_approx_fast(
                d["ri"][s][:, n:n + 1], d["r"][s][:, n:n + 1])

    def av_vsc(p, m):
        # dep-free by emission time: recip(m) was emitted a step earlier
        d = st8[p]
        if d["o"] is None:
            d["o"] = o_pool.tile([128, S], F32, tag="o", name=f"o_{p}")
        out = []
        for s in AB:
            if no_vsc:
                vsc = d["v"][s][:, m * 64:(m + 1) * 64]
            else:
                vsc = vsc_pool.tile([128, 64], BF16, tag=f"vsc{s}",
                                    name=f"vsc{s}_{p}")
                nc.vector.tensor_scalar_mul(
                    vsc[:], d["v"][s][:, m * 64:(m + 1) * 64],
                    d["ri"][s][:, m:m + 1])
            out.append(vsc)
        return out

    def av_quarter(p, m, vscs, s, chs):
        # 2 of the 8 AV matmuls for k-tile m: head s, chunk pair chs
        d = st8[p]
        for ch in chs:
            cs = slice(ch * 512, (ch + 1) * 512)
            nc.tensor.matmul(
                d["o"][64 * s:64 * s + 64, cs],
                lhsT=vscs[s][:],
                rhs=d["e"][s][m][:, cs],
                start=True if av_noacc else (m == 0),
                stop=True if av_noacc else (m == NT - 1),
                skip_group_check=True,
            )

    def pair_end(p):
        if no_out:
            return
        d = st8[p]
        out_sb = osb_pool.tile([128, S], F16, tag="osb", name=f"osb_{p}")
        for h in range(4):
            cs = slice(h * 512, (h + 1) * 512)
            nc.vector.tensor_copy(out_sb[:, cs], d["o"][:, cs])
        nc.gpsimd.dma_start(ot[d["bh"][0]], out_sb[0:64, :])
        nc.sync.dma_start(ot[d["bh"][1]], out_sb[64:128, :])

    # flat software pipeline: global QK stream with AV trailing by av_lag,
    # flowing across pair boundaries
    total = NP * NT
    for g in range(total + av_lag):
        if g >= 1 and g - 1 < total and not skip_exp:
            pr, nr = divmod(g - 1, NT)
            r_chain(pr, nr)
        do_av = g >= av_lag and not (skip_av or skip_exp)
        if do_av:
            pm, mm = divmod(g - av_lag, NT)
            vscs = av_vsc(pm, mm)
        if g < total:
            p, n = divmod(g, NT)
            if n == 0:
                pair_start(p)
            for s in AB:
                er = er_pool.tile([128, S], BF16, tag=f"er{s}",
                                  name=f"er{s}_{p}_{n}")
                st8[p]["e"][s][n] = er
            if cfg.get("fine", True):
                # fine-grained interleave: 2 AV matmuls after each QK half so
                # the in-order PE stream never delays an st fill by ~>400ns
                for i, (s, h) in enumerate(((0, 0), (0, 1), (1, 0), (1, 1))):
                    qk_half(p, n, s, h)
                    if do_av:
                        av_quarter(pm, mm, vscs, i // 2, (2 * (i % 2),
                                                          2 * (i % 2) + 1))
            else:
                # coarse: QK for both heads, then the full AV unit
                for s, h in ((0, 0), (0, 1), (1, 0), (1, 1)):
                    qk_half(p, n, s, h)
                if do_av:
                    for s in AB:
                        av_quarter(pm, mm, vscs, s, (0, 1))
                        av_quarter(pm, mm, vscs, s, (2, 3))
        elif do_av:
            for s in AB:
                av_quarter(pm, mm, vscs, s, (0, 1))
                av_quarter(pm, mm, vscs, s, (2, 3))
        if do_av and mm == NT - 1:
            pair_end(pm)


_NC_CACHE = {}


def build_nc(repeats=1, **cfg):
    key = (repeats, tuple(sorted(cfg.items())))
    if key in _NC_CACHE:
        return _NC_CACHE[key]
    nc = bacc.Bacc("TRN2", target_bir_lowering=False, debug=False)
    qt = nc.dram_tensor("qt", [NP, 2 * D, S], F16, kind="ExternalInput").ap()
    kt = nc.dram_tensor("kt", [NP, 2 * D, S], F16, kind="ExternalInput").ap()
    vs = nc.dram_tensor("vs", [BH_PER_CORE, 128, NT * 64], BF16, kind="ExternalInput").ap()
    rg = nc.dram_tensor("rg", [128, 128], F32, kind="ExternalInput").ap()
    rgb = nc.dram_tensor("rgb", [128, 256], BF16, kind="ExternalInput").ap()
    ot = nc.dram_tensor("ot", [BH_PER_CORE, D, S], F16, kind="ExternalOutput").ap()
    with tile.TileContext(nc) as tc, ExitStack() as ctx:
        if repeats == 1:
            _build_kernel(nc, tc, ctx, qt, kt, vs, rg, rgb, ot, cfg)
        else:
            # benchmarking mode: repeat the whole kernel body in an on-device
            # loop so per-iteration time can be extracted from wall clock
            with tc.For_i(0, repeats, 1,
                          staggered_reset=cfg.get("stag", True),
                          hint_engines=(mybir.EngineType.PE,
                                        mybir.EngineType.Activation,
                                        mybir.EngineType.DVE)):
                _build_kernel(nc, tc, ctx, qt, kt, vs, rg, rgb, ot, cfg)
    nc.compile()
    _NC_CACHE[key] = nc
    return nc


def _prep_inputs(qry, key, val, reg):
    """Host-side shard + layout prep. Returns per-core input maps."""
    q = np.ascontiguousarray(np.asarray(qry, dtype=np.float32)).reshape(BH, S, D)
    k = np.ascontiguousarray(np.asarray(key, dtype=np.float32)).reshape(BH, S, D)
    v = np.ascontiguousarray(np.asarray(val, dtype=np.float32)).reshape(BH, S, D)
    rg = (np.eye(128, dtype=np.float32) * np.float32(np.asarray(reg)))
    rgb = np.concatenate([np.eye(128, dtype=np.float32), rg],
                         axis=1).astype(ml_dtypes.bfloat16)

    in_maps = []
    for c in range(N_CORES):
        sl = slice(c * BH_PER_CORE, (c + 1) * BH_PER_CORE)
        qt = np.ascontiguousarray(
            q[sl].transpose(0, 2, 1).reshape(NP, 2 * D, S)
        ).astype(np.float16)                                          # [2, 128, S]
        kt = np.ascontiguousarray(
            k[sl].transpose(0, 2, 1).reshape(NP, 2 * D, S)
        ).astype(np.float16)                                          # [2, 128, S]
        vv = v[sl].reshape(BH_PER_CORE, NT, 128, D)
        vs = np.ascontiguousarray(vv.transpose(0, 2, 1, 3)).reshape(
            BH_PER_CORE, 128, NT * D).astype(ml_dtypes.bfloat16)      # [4, 128, 1024]
        in_maps.append({"qt": qt, "kt": kt, "vs": vs, "rg": rg,
                        "rgb": rgb})
    return in_maps


def kernel(qry, key, val, reg):
    nc = build_nc()
    in_maps = _prep_inputs(qry, key, val, reg)
    res = run_bass_kernel_spmd(nc, in_maps, list(range(N_CORES)))
    out = np.empty((BH, S, D), dtype=np.float32)
    for c in range(N_CORES):
        ot = res.results[c]["ot"].astype(np.float32)                 # [4, 64, S]
        for i in range(BH_PER_CORE):
            out[c * BH_PER_CORE + i] = ot[i].T
    return out.reshape(B, H, S, D)
